# revision 14
# baseline (speedup 1.0000x reference)
"""Trainium2 Bass kernel for nn_Decoder_4561255269164 (retrieval_knn).

Math: the reference's top-K(8) KNN collapses to min-reductions:
  - backward: weight w=1/sqrt(d) is nonzero only where d equals the row min
    (over kept candidates), so the scatter-add num/den equals
    E_b^T @ [w*rgb, w] with E_b[i,j] = (d2[i,j] == rowmin_i).
  - forward: only the column argmin rows of d2 matter; sf/cntf =
    E_f^T @ [rgb, 1] with E_f[i,j] = (d2[i,j] <= colmin_j).
  - exact-match (d==0) rows use a separate weight column gated by rowmin==0.

Key optimizations over the fp32 3-pass version:
  - Only KEPT candidates (exactly points_num = 8192 of 16384) participate in
    the whole recolor loss, so candidates are compacted host-side to KC=8192
    columns. Halves every pass.
  - d2 is computed in bf16 matmuls (1 cycle/col vs fp32's 4) with hi/lo split
    coordinates (16 contract rows) for ~1e-2 absolute accuracy; both passes
    compute bitwise-identical values so equality compares need no epsilon.
  - Pass A computes row mins AND column mins in one sweep: the Act engine
    copies PSUM to negated bf16, DVE max-folds rows and columns, gpsimd
    partition_all_reduce(max) collapses partitions for the column mins.
  - Column-min AllReduce fires in two halves early (jcg-outer loop) so it
    hides under pass A/B compute.
  - Scatter matmuls run concurrently via col-group tile_position, distance
    matmuls via row-group tile_position packing.
  - nd reduction uses ReduceScatter + per-core sharded finalize; the host
    sums the 8 per-core partial [coord_loss, rgb_loss] outputs.
"""

import numpy as np

import concourse.bass as bass
import concourse.bass_isa as bass_isa
import concourse.bacc as bacc
import concourse.mybir as mybir
import concourse.tile as tile
from concourse import library_config
from concourse.bass_utils import run_bass_kernel_spmd

F32 = mybir.dt.float32
BF16 = mybir.dt.bfloat16
AX = mybir.AxisListType
ALU = mybir.AluOpType
ACTF = mybir.ActivationFunctionType
NPBF = mybir.dt.np(BF16)

# geometry
NCORES = 8
L = 16384          # candidate voxels (full)
N = 10000          # targets
NSH = N // NCORES  # targets per core (1250)
NT = 10            # i-tiles of 128 per core (pad 1250 -> 1280)
NPAD = NT * 128
KC = 8192          # compacted (kept) candidate columns = points_num
SHW = KC // NCORES  # finalize shard width per core (1024)
LSH = L // NCORES   # BCE shard width per core (2048)
BIG = np.float32(1e30)

CWA = 2048         # pass A chunk width (4 x 512 row-group-packed matmuls)
GA = 4
NJA = KC // CWA    # 4
CWB = 1024         # pass B chunk width (2 x 512)
GB = 2
NJB = KC // CWB    # 8
NMETA = 4          # pass B meta-passes (2 j-groups each, PSUM-limited)


def _build_nc(reps=1, phases=("A", "W", "B", "FIN")):
    nc = bacc.Bacc("TRN2", target_bir_lowering=False, debug=False,
                   num_devices=NCORES)

    c16d = nc.declare_dram_parameter("c16", [16, KC], BF16, isOutput=False)
    t16d = nc.declare_dram_parameter("t16", [16, NPAD], BF16, isOutput=False)
    trgbd = nc.declare_dram_parameter("trgb", [128, NT * 3], F32,
                                      isOutput=False)
    rgbshd = nc.declare_dram_parameter("rgbsh", [3, SHW], F32, isOutput=False)
    keepshd = nc.declare_dram_parameter("keepsh", [1, SHW], F32,
                                        isOutput=False)
    predshd = nc.declare_dram_parameter("predsh", [1, LSH], F32,
                                        isOutput=False)
    ktgtshd = nc.declare_dram_parameter("ktgtsh", [1, LSH], F32,
                                        isOutput=False)
    chaind = nc.declare_dram_parameter("chain", [1, 2], F32, isOutput=False)
    outd = nc.declare_dram_parameter("out", [1, 2], F32, isOutput=True)

    rg = [list(range(NCORES))]

    with tile.TileContext(nc) as tc:
        nc.gpsimd.load_library(library_config.mlp)
        for _rep in range(reps):
            with (
                tc.tile_pool(name="persist", bufs=1) as pp,
                tc.tile_pool(name="dram", bufs=1, space="DRAM") as dp,
            ):
                # ---- persistent SBUF state ----
                Cr = pp.tile([(GA - 1) * 32 + 16, KC], BF16, tag="Cr",
                             name="Cr")
                T5r = pp.tile([(GA - 1) * 32 + 16, NPAD], BF16, tag="T5r",
                              name="T5r")
                for g in range(GA):
                    nc.sync.dma_start(Cr[32 * g:32 * g + 16, :], c16d[:, :])
                    nc.sync.dma_start(T5r[32 * g:32 * g + 16, :], t16d[:, :])
                trgbs = pp.tile([128, NT * 3], F32, tag="trgbs", name="trgbs")
                nc.sync.dma_start(trgbs[:], trgbd[:, :])

                colneg = pp.tile([128, KC], BF16, tag="colneg", name="colneg")
                rowneg = pp.tile([128, NT * 1024], BF16, tag="rowneg",
                                 name="rowneg")
                # negated colmin broadcast for cached groups (cols 0:CWA),
                # positive for the rest
                m2negb = pp.tile([128, CWA], BF16, tag="m2negb",
                                 name="m2negb")
                m2posb = pp.tile([128, KC - CWA], BF16, tag="m2posb",
                                 name="m2posb")
                m_bf = pp.tile([128, NT], F32, tag="m_bf")
                mneg = pp.tile([128, NT], F32, tag="mneg")
                wb_all = pp.tile([128, NT * 8], BF16, tag="wb_all")
                wf_all = pp.tile([128, NT * 4], BF16, tag="wf_all")
                # pass-A d2b cache (negated) for columns 0:CWA, all i-tiles
                ndc = pp.tile([128, NT * CWA], BF16, tag="ndc", name="ndc")

                m2i = [dp.tile([1, 2 * CWA], BF16, tag=f"m2i{h}",
                               name=f"m2i{h}") for h in range(2)]
                m2o = [dp.tile([1, 2 * CWA], BF16, tag=f"m2o{h}",
                               name=f"m2o{h}") for h in range(2)]
                ndi = dp.tile([NJB * 12, SHW], BF16, tag="ndi", name="ndi")
                ndo = dp.tile([12, SHW], BF16, tag="ndo", name="ndo")

                if "A" in phases:
                    # ------- pass A: d2 sweep -> row mins + col mins -------
                    nc.vector.memset(rowneg[:], -float(BIG))
                    with (
                        tc.tile_pool(name="a_ps", bufs=2, space="PSUM") as psa,
                        tc.tile_pool(name="a_nd", bufs=3) as nda,
                        tc.tile_pool(name="a_cm", bufs=2) as cma,
                        tc.tile_pool(name="a_row", bufs=1) as rpa,
                    ):
                        for jcg in range(NJA):
                            j0 = jcg * CWA
                            for t in range(NT):
                                ps = psa.tile([128, CWA], F32, tag="psA")
                                for g in range(GA):
                                    nc.tensor.matmul(
                                        ps[:, 512 * g:512 * (g + 1)],
                                        lhsT=T5r[32 * g:32 * g + 16,
                                                 128 * t:128 * (t + 1)],
                                        rhs=Cr[32 * g:32 * g + 16,
                                               j0 + 512 * g:j0 + 512 * (g + 1)],
                                        start=True, stop=True,
                                        tile_position=(32 * g, 0))
                                if jcg == 0:
                                    base = t * CWA

                                    def nd2s(a, b, base=base):
                                        return ndc[:, base + a:base + b]
                                else:
                                    ntl = nda.tile([128, CWA], BF16,
                                                   tag="nd2")

                                    def nd2s(a, b, ntl=ntl):
                                        return ntl[:, a:b]
                                nc.scalar.activation(nd2s(0, CWA), ps[:],
                                                     ACTF.Copy, scale=-1.0)
                                rsl = rowneg[:, t * 1024:(t + 1) * 1024]
                                nc.vector.tensor_tensor(
                                    rsl, rsl, nd2s(0, 1024), op=ALU.max)
                                nc.vector.tensor_tensor(
                                    rsl, rsl, nd2s(1024, 2048), op=ALU.max)
                                csl = colneg[:, j0:j0 + CWA]
                                if t == 0:
                                    nc.vector.tensor_copy(csl, nd2s(0, CWA))
                                else:
                                    nc.vector.tensor_tensor(
                                        csl, csl, nd2s(0, CWA), op=ALU.max)
                            # col-min (negated -> max) across partitions
                            cm = cma.tile([128, CWA], BF16, tag="cmA")
                            nc.gpsimd.partition_all_reduce(
                                cm[:], colneg[:, j0:j0 + CWA], 128,
                                bass_isa.ReduceOp.max)
                            half, piece = jcg // 2, jcg % 2
                            nc.sync.dma_start(
                                m2i[half][0:1, piece * CWA:(piece + 1) * CWA],
                                cm[0:1, :])
                            if piece == 1:
                                if NCORES > 1 and "NOAR" not in phases:
                                    nc.gpsimd.collective_compute(
                                        "AllReduce", ALU.max,
                                        replica_groups=rg,
                                        ins=[m2i[half].opt()],
                                        outs=[m2o[half].opt()])
                                else:
                                    nc.sync.dma_start(m2o[half][:, :],
                                                      m2i[half][:, :])
                                row = rpa.tile([1, 2 * CWA], BF16,
                                               tag=f"m2r{half}",
                                               name=f"m2r{half}")
                                nc.sync.dma_start(row[:], m2o[half][:, :])
                                if half == 0:
                                    # cols 0:CWA stay negated (cached cmp)
                                    nc.gpsimd.partition_broadcast(
                                        m2negb[:, :], row[0:1, 0:CWA])
                                    rowp = rpa.tile([1, CWA], BF16,
                                                    tag="m2p0", name="m2p0")
                                    nc.vector.tensor_scalar(
                                        rowp[:], row[0:1, CWA:2 * CWA], -1.0,
                                        None, op0=ALU.mult)
                                    nc.gpsimd.partition_broadcast(
                                        m2posb[:, 0:CWA], rowp[:])
                                else:
                                    rowp = rpa.tile([1, 2 * CWA], BF16,
                                                    tag="m2p1", name="m2p1")
                                    nc.vector.tensor_scalar(
                                        rowp[:], row[:], -1.0, None,
                                        op0=ALU.mult)
                                    nc.gpsimd.partition_broadcast(
                                        m2posb[:, CWA:3 * CWA], rowp[:])

                if "W" in phases:
                    # ------- row-min finalize + weight tiles -------
                    with tc.tile_pool(name="wsmall", bufs=1) as ws:
                        for t in range(NT):
                            nc.vector.tensor_reduce(
                                mneg[:, t:t + 1],
                                rowneg[:, t * 1024:(t + 1) * 1024],
                                axis=AX.X, op=ALU.max)
                        m_all = ws.tile([128, NT], F32, tag="m_all")
                        nc.vector.tensor_scalar(m_all[:], mneg[:], -1.0,
                                                None, op0=ALU.mult)
                        nc.vector.tensor_copy(m_bf[:], m_all[:])
                        m_relu = ws.tile([128, NT], F32, tag="m_relu")
                        nc.vector.tensor_scalar(m_relu[:], m_all[:], 0.0,
                                                None, op0=ALU.max)
                        msafe = ws.tile([128, NT], F32, tag="msafe")
                        nc.vector.tensor_scalar(msafe[:], m_relu[:], 1e-30,
                                                None, op0=ALU.max)
                        sqm = ws.tile([128, NT], F32, tag="sqm")
                        nc.scalar.activation(sqm[:], msafe[:], ACTF.Sqrt)
                        w0 = ws.tile([128, NT], F32, tag="w0")
                        nc.vector.reciprocal(w0[:], sqm[:])
                        vv = ws.tile([128, NT], F32, tag="vv")
                        nc.vector.tensor_scalar(vv[:], m_relu[:], 0.0, None,
                                                op0=ALU.is_gt)
                        v2 = ws.tile([128, NT], F32, tag="v2")
                        nc.vector.tensor_scalar(v2[:], m_relu[:], 1e29, None,
                                                op0=ALU.is_lt)
                        nc.vector.tensor_tensor(vv[:], vv[:], v2[:],
                                                op=ALU.mult)
                        wgt = ws.tile([128, NT], F32, tag="wgt")
                        nc.vector.tensor_tensor(wgt[:], w0[:], vv[:],
                                                op=ALU.mult)
                        zz = ws.tile([128, NT], F32, tag="zz")
                        nc.vector.tensor_scalar(zz[:], m_relu[:], 0.0, None,
                                                op0=ALU.is_equal)

                        wbv = wb_all[:].rearrange("p (t k) -> p t k", k=8)
                        wfv = wf_all[:].rearrange("p (t k) -> p t k", k=4)
                        tv = trgbs[:].rearrange("p (t k) -> p t k", k=3)
                        wgv = wgt[:].rearrange("p (t o) -> p t o", o=1)
                        zzv = zz[:].rearrange("p (t o) -> p t o", o=1)
                        for c in range(3):
                            nc.vector.tensor_tensor(
                                wbv[:, :, c:c + 1], wgv, tv[:, :, c:c + 1],
                                op=ALU.mult)
                            nc.vector.tensor_tensor(
                                wbv[:, :, 4 + c:5 + c], zzv, tv[:, :, c:c + 1],
                                op=ALU.mult)
                            nc.vector.tensor_copy(wfv[:, :, c:c + 1],
                                                  tv[:, :, c:c + 1])
                        nc.vector.tensor_copy(wbv[:, :, 3:4], wgv)
                        nc.vector.tensor_copy(wbv[:, :, 7:8], zzv)
                        nc.vector.memset(wfv[:, :, 3:4], 1.0)

                if "B" in phases:
                    # ------- pass B: indicators + scatter matmuls -------
                    with (
                        tc.tile_pool(name="b_ps", bufs=2, space="PSUM") as psb,
                        tc.tile_pool(name="b_acc", bufs=1,
                                     space="PSUM") as accp,
                        tc.tile_pool(name="b_d2", bufs=3) as dbp,
                        tc.tile_pool(name="b_e", bufs=4) as ebp,
                    ):
                        for meta in range(NMETA):
                            accs = [accp.tile([36, CWB], F32, tag=f"acc{q}",
                                              name=f"acc{q}")
                                    for q in range(2)]
                            for t in range(NT):
                                for q in range(2):
                                    jc = meta * 2 + q
                                    j0 = jc * CWB
                                    if j0 < CWA:
                                        # cached negated d2b from pass A
                                        off = t * CWA + j0
                                        src = ndc[:, off:off + CWB]
                                        eb = ebp.tile([128, CWB], BF16,
                                                      tag="eb")
                                        nc.vector.tensor_scalar(
                                            eb[:], src, mneg[:, t:t + 1],
                                            None, op0=ALU.is_equal)
                                        ef = ebp.tile([128, CWB], BF16,
                                                      tag="ef")
                                        nc.vector.tensor_tensor(
                                            ef[:], src, m2negb[:, j0:j0 + CWB],
                                            op=ALU.is_ge)
                                    else:
                                        ps = psb.tile([128, CWB], F32,
                                                      tag="psB")
                                        for g in range(GB):
                                            nc.tensor.matmul(
                                                ps[:, 512 * g:512 * (g + 1)],
                                                lhsT=T5r[32 * g:32 * g + 16,
                                                         128 * t:
                                                         128 * (t + 1)],
                                                rhs=Cr[32 * g:32 * g + 16,
                                                       j0 + 512 * g:
                                                       j0 + 512 * (g + 1)],
                                                start=True, stop=True,
                                                tile_position=(32 * g, 0))
                                        d2b = dbp.tile([128, CWB], BF16,
                                                       tag="d2b")
                                        nc.scalar.activation(d2b[:], ps[:],
                                                             ACTF.Copy)
                                        eb = ebp.tile([128, CWB], BF16,
                                                      tag="eb")
                                        nc.vector.tensor_scalar(
                                            eb[:], d2b[:], m_bf[:, t:t + 1],
                                            None, op0=ALU.is_equal)
                                        ef = ebp.tile([128, CWB], BF16,
                                                      tag="ef")
                                        nc.vector.tensor_tensor(
                                            ef[:], d2b[:],
                                            m2posb[:, j0 - CWA:
                                                   j0 - CWA + CWB],
                                            op=ALU.is_le)
                                    for h in range(2):
                                        hs = slice(512 * h, 512 * (h + 1))
                                        nc.tensor.matmul(
                                            accs[q][0:8, hs],
                                            lhsT=wb_all[:, 8 * t:8 * (t + 1)],
                                            rhs=eb[:, hs],
                                            start=(t == 0),
                                            stop=(t == NT - 1),
                                            tile_position=(0, 0))
                                        nc.tensor.matmul(
                                            accs[q][32:36, hs],
                                            lhsT=wf_all[:, 4 * t:4 * (t + 1)],
                                            rhs=ef[:, hs],
                                            start=(t == 0),
                                            stop=(t == NT - 1),
                                            tile_position=(0, 32))
                            for q in range(2):
                                jc = meta * 2 + q
                                ndsb = ebp.tile([36, CWB], BF16, tag="ndsb",
                                                name="ndsb")
                                nc.scalar.copy(ndsb[:], accs[q][:])
                                nc.sync.dma_start(
                                    ndi[jc * 12:jc * 12 + 8, :],
                                    ndsb[0:8, :])
                                nc.sync.dma_start(
                                    ndi[jc * 12 + 8:jc * 12 + 12, :],
                                    ndsb[32:36, :])
                    if NCORES > 1 and "NOAR" not in phases:
                        nc.gpsimd.collective_compute(
                            "ReduceScatter", ALU.add, replica_groups=rg,
                            ins=[ndi.opt()], outs=[ndo.opt()])
                    else:
                        nc.sync.dma_start(ndo[:, :], ndi[0:12, :])

                if "FIN" in phases:
                    # ------- per-core shard finalize -------
                    lp = SHW // 128   # 8
                    lp2 = LSH // 128  # 16
                    with (
                        tc.tile_pool(name="fin", bufs=1) as fp,
                        tc.tile_pool(name="fin_ps", bufs=1,
                                     space="PSUM") as fps,
                    ):
                        def plane_from(dram_row, tg, w, dt=F32):
                            tl = fp.tile([128, w], dt, tag=tg, name=tg)
                            nc.sync.dma_start(
                                tl[:], dram_row.rearrange("(p q) -> p q",
                                                          p=128))
                            if dt is not F32:
                                tf = fp.tile([128, w], F32, tag=tg + "f",
                                             name=tg + "f")
                                nc.vector.tensor_copy(tf[:], tl[:])
                                return tf
                            return tl

                        nd = [plane_from(ndo[k, :], f"nd{k}", lp, BF16)
                              for k in range(12)]
                        rgbp = [plane_from(rgbshd[k, :], f"rgb{k}", lp)
                                for k in range(3)]
                        keepf = plane_from(keepshd[0, :], "keepf", lp)
                        predf = plane_from(predshd[0, :], "predf", lp2)
                        ktgt = plane_from(ktgtshd[0, :], "ktgt", lp2)

                        num, den = nd[0:3], nd[3]
                        s0, cnt0 = nd[4:7], nd[7]
                        sf, cntf = nd[8:11], nd[11]

                        _cnt = [0]

                        def newt(w=lp):
                            _cnt[0] += 1
                            return fp.tile([128, w], F32,
                                           tag=f"fin{_cnt[0]}",
                                           name=f"fin{_cnt[0]}")

                        dsafe = newt()
                        nc.vector.tensor_scalar(dsafe[:], den[:], 0.0, None,
                                                op0=ALU.is_equal)
                        nc.vector.tensor_tensor(dsafe[:], dsafe[:], den[:],
                                                op=ALU.add)
                        rden = newt()
                        nc.vector.reciprocal(rden[:], dsafe[:])
                        c0safe = newt()
                        nc.vector.tensor_scalar(c0safe[:], cnt0[:], 0.0, None,
                                                op0=ALU.is_equal)
                        nc.vector.tensor_tensor(c0safe[:], c0safe[:],
                                                cnt0[:], op=ALU.add)
                        rcnt0 = newt()
                        nc.vector.reciprocal(rcnt0[:], c0safe[:])
                        cfsafe = newt()
                        nc.vector.tensor_scalar(cfsafe[:], cntf[:], 0.0, None,
                                                op0=ALU.is_equal)
                        nc.vector.tensor_tensor(cfsafe[:], cfsafe[:],
                                                cntf[:], op=ALU.add)
                        rcntf = newt()
                        nc.vector.reciprocal(rcntf[:], cfsafe[:])

                        mden = fp.tile([128, lp], mybir.dt.int32, tag="mden",
                                       name="mden")
                        nc.vector.tensor_scalar(mden[:], den[:], 0.0, None,
                                                op0=ALU.not_equal)
                        mz = fp.tile([128, lp], mybir.dt.int32, tag="mz",
                                     name="mz")
                        nc.vector.tensor_scalar(mz[:], cnt0[:], 0.0, None,
                                                op0=ALU.is_gt)

                        acc = newt()
                        nc.vector.memset(acc[:], 0.0)
                        for c in range(3):
                            rec = newt()
                            nc.vector.tensor_tensor(rec[:], sf[c][:],
                                                    rcntf[:], op=ALU.mult)
                            tmp = newt()
                            nc.vector.tensor_tensor(tmp[:], num[c][:],
                                                    rden[:], op=ALU.mult)
                            nc.vector.copy_predicated(rec[:], mden[:], tmp[:])
                            nc.vector.tensor_tensor(tmp[:], s0[c][:],
                                                    rcnt0[:], op=ALU.mult)
                            nc.vector.copy_predicated(rec[:], mz[:], tmp[:])
                            diff = newt()
                            nc.vector.tensor_tensor(diff[:], rgbp[c][:],
                                                    rec[:], op=ALU.subtract)
                            ad = newt()
                            nc.scalar.activation(ad[:], diff[:], ACTF.Abs)
                            nc.vector.tensor_tensor(acc[:], acc[:], ad[:],
                                                    op=ALU.add)
                        nc.vector.tensor_tensor(acc[:], acc[:], keepf[:],
                                                op=ALU.mult)

                        # BCE: relu(p) - p*t + softplus(-|p|)
                        bce = newt(lp2)
                        nc.scalar.activation(bce[:], predf[:], ACTF.Relu)
                        pt = newt(lp2)
                        nc.vector.tensor_tensor(pt[:], predf[:], ktgt[:],
                                                op=ALU.mult)
                        nc.vector.tensor_tensor(bce[:], bce[:], pt[:],
                                                op=ALU.subtract)
                        ap_ = newt(lp2)
                        nc.scalar.activation(ap_[:], predf[:], ACTF.Abs)
                        en = newt(lp2)
                        nc.scalar.activation(en[:], ap_[:], ACTF.Exp,
                                             scale=-1.0)
                        sp = newt(lp2)
                        nc.scalar.activation(sp[:], en[:], ACTF.Ln, bias=1.0)
                        nc.vector.tensor_tensor(bce[:], bce[:], sp[:],
                                                op=ALU.add)

                        rows2 = fp.tile([128, 2], F32, tag="rows2")
                        nc.vector.tensor_reduce(rows2[:, 0:1], bce[:],
                                                axis=AX.X, op=ALU.add)
                        nc.vector.tensor_reduce(rows2[:, 1:2], acc[:],
                                                axis=AX.X, op=ALU.add)
                        onescol = fp.tile([128, 1], F32, tag="onescol")
                        nc.vector.memset(onescol[:], 1.0)
                        pstot = fps.tile([1, 2], F32, tag="pstot")
                        nc.tensor.matmul(pstot[:], lhsT=onescol[:],
                                         rhs=rows2[:], start=True, stop=True)
                        chsb = fp.tile([1, 2], F32, tag="chsb")
                        nc.sync.dma_start(chsb[:], chaind[:, :])
                        nc.vector.tensor_scalar(chsb[:], chsb[:], 0.0, None,
                                                op0=ALU.mult)
                        outsb = fp.tile([1, 2], F32, tag="outsb")
                        nc.scalar.copy(outsb[:], pstot[:])
                        nc.vector.tensor_tensor(outsb[:], outsb[:], chsb[:],
                                                op=ALU.add)
                        nc.sync.dma_start(outd[:, :], outsb[:])

    nc.compile()
    return nc


def _bfsplit(x):
    """f32 array -> (hi, lo) bf16 arrays with hi + lo ~= x."""
    x = np.asarray(x, np.float32)
    hi = x.astype(NPBF)
    lo = (x - hi.astype(np.float32)).astype(NPBF)
    return hi, lo


def _host_prep(pred_F, cand_xyz, cand_rgb, tgt_xyz, tgt_rgb, keep_target,
               points_num):
    pred = np.ascontiguousarray(np.asarray(pred_F, np.float32))
    cxyz = np.ascontiguousarray(np.asarray(cand_xyz, np.float32))
    crgb = np.ascontiguousarray(np.asarray(cand_rgb, np.float32))
    txyz = np.ascontiguousarray(np.asarray(tgt_xyz, np.float32))
    trgb_np = np.ascontiguousarray(np.asarray(tgt_rgb, np.float32))
    ktgt = np.asarray(keep_target).astype(np.float32)

    # keep mask (exact reference semantics)
    p8 = pred.reshape(-1, 8)
    rows = np.arange(p8.shape[0])
    ilm = np.zeros(p8.shape, dtype=bool)
    ilm[rows, np.argmax(p8, axis=1)] = True
    ilm = ilm.reshape(-1)
    k = L - int(points_num)
    vals = np.where(ilm, np.inf, pred)
    thr = np.sort(vals)[k - 1]
    keep = (pred > thr) | ilm

    kidx = np.nonzero(keep)[0]
    nk = len(kidx)
    if nk > KC:  # only possible with pred ties; drop extras (tiny loss shift)
        kidx = kidx[:KC]
        nk = KC

    cx = cxyz[kidx]                       # [nk, 3]
    ch, cl = _bfsplit(cx)
    b2 = np.sum(cx * cx, axis=1, dtype=np.float32).astype(np.float32)
    bh, bl = _bfsplit(b2)
    C = np.zeros((16, KC), NPBF)
    C[0:3, :nk] = ch.T
    C[3:6, :nk] = ch.T
    C[6:9, :nk] = cl.T
    C[9:12, :nk] = cl.T
    C[12, :nk] = np.float32(1.0)
    C[13, :nk] = np.float32(1.0)
    C[14, :nk] = bh
    C[15, :nk] = bl
    C[14, nk:] = BIG  # pad columns: s = 1e30 (row 14 pairs with T=ones)

    rgbk = np.zeros((3, KC), np.float32)
    rgbk[:, :nk] = (crgb[kidx] * np.float32(255.0)).T
    keepk = np.zeros((1, KC), np.float32)
    keepk[0, :nk] = 1.0

    common = dict(c16=np.ascontiguousarray(C),
                  chain=np.zeros((1, 2), np.float32))

    in_maps = []
    for c in range(NCORES):
        sl = slice(c * NSH, (c + 1) * NSH)
        tc_ = txyz[sl]
        th, tl = _bfsplit(tc_)
        a2 = np.sum(tc_ * tc_, axis=1, dtype=np.float32).astype(np.float32)
        ah, al = _bfsplit(a2)
        T = np.zeros((16, NPAD), NPBF)
        T[0:3, :NSH] = (-2.0 * th.astype(np.float32)).astype(NPBF).T
        T[3:6, :NSH] = (-2.0 * tl.astype(np.float32)).astype(NPBF).T
        T[6:9, :NSH] = T[0:3, :NSH]
        T[9:12, :NSH] = T[3:6, :NSH]
        T[12, :NSH] = ah
        T[13, :NSH] = al
        T[14, :NSH] = np.float32(1.0)
        T[15, :NSH] = np.float32(1.0)
        T[12, NSH:] = BIG  # pad target rows: s = 1e30 everywhere

        tr = np.zeros((NPAD, 3), np.float32)
        tr[:NSH] = trgb_np[sl]
        trc = tr.reshape(NT, 128, 3).transpose(1, 0, 2).reshape(128, NT * 3)

        ssl = slice(c * SHW, (c + 1) * SHW)
        lsl = slice(c * LSH, (c + 1) * LSH)
        in_maps.append(dict(
            common,
            t16=np.ascontiguousarray(T),
            trgb=np.ascontiguousarray(trc),
            rgbsh=np.ascontiguousarray(rgbk[:, ssl]),
            keepsh=np.ascontiguousarray(keepk[:, ssl]),
            predsh=np.ascontiguousarray(pred[lsl].reshape(1, LSH)),
            ktgtsh=np.ascontiguousarray(ktgt[lsl].reshape(1, LSH)),
        ))
    return in_maps


_CACHE = {}


def kernel(pred_F, cand_xyz, cand_rgb, tgt_xyz, tgt_rgb, keep_target,
           points_num=8192, **_ignored):
    in_maps = _host_prep(pred_F, cand_xyz, cand_rgb, tgt_xyz, tgt_rgb,
                         keep_target, points_num)
    if "nc" not in _CACHE:
        _CACHE["nc"] = _build_nc()
    res = run_bass_kernel_spmd(_CACHE["nc"], in_maps,
                               core_ids=list(range(NCORES)))
    tot = np.zeros(2, np.float32)
    for c in range(NCORES):
        tot += np.asarray(res.results[c]["out"], np.float32).reshape(2)
    return tot


if __name__ == "__main__":
    import reference as R
    inputs = R.setup_inputs()
    inputs = {kk: np.asarray(vv) if not np.isscalar(vv) else vv
              for kk, vv in inputs.items()}
    out = kernel(**inputs)
    print("kernel out:", out)
